# revision 20
# baseline (speedup 1.0000x reference)
"""BiAttention (BiDAF-style) Trainium2 kernel — 8-core SPMD, memory-bound.

Contract: kernel(**inputs) takes the FULL tensors
  text [32,8,512,128] f32, query [32,64,128] f32, text_mask [32,8,512],
  query_mask [32,64], w [384], b [1]
and returns attn [32,8,512,512] f32, matching the reference

  w1,w2,w3 = w[:128], w[128:256], w[256:]
  logits[b,m,i,j] = text[b,m,i]·(w3*query[b,j]) + t1[b,m,i] + q2[b,j] + b
  p_q   = softmax_j logits      -> query_attn = p_q @ query
  qlmax = max_j logits          -> p_text = softmax_i qlmax
  text_attn = sum_i p_text*text
  out = concat([text, query_attn, text*query_attn, text*text_attn], -1)

The masks are all ones per the problem spec, so the (1-mask)*VERY_NEG term is
identically zero; bias b and the per-row t1 shift cancel inside softmax_j
(t1 is carried exactly via an extra lhsT column for the qlmax path).

v3 — HBM-traffic-minimized (16.9 MB/core vs 41.9 baseline):
  * output block 0 is a verbatim copy of the input `text`; the host fills it
    during unshard; the device stores only [query_attn, text*query_attn,
    text*text_attn] in fp16.
  * text is loaded bf16 with a paired-row interleave (partition p holds rows
    {256t+2p, 256t+2p+1}) keeping every DMA descriptor >= 512 B contiguous.
  * all matmuls bf16 (1 PE col/cycle).
  * qn carries an appended ones column so the attnu matmul also emits the
    softmax_j denominators Z(i) per-partition.
  * text_attn: stride-0-broadcast stationary (every PE column = etq) makes
    the weighted-sum matmul emit its result broadcast across all 128
    partitions; one fused DVE tensor_scalar normalizes+casts it to bf16.
  * elementwise work is split across DVE / ACT / Pool per CFG.

Sharding: batch B=32 data-parallel across 8 NeuronCores (BLOC=4 per core),
32 (b,m) units per core, no collectives.  Host precomputes O(query)-sized
helpers only (packed into one bf16 tile per batch + tiny f32 q2).

Toolchain note: walrus in this container encodes ONE sync-wait per
instruction; _split_multi_waits() legalizes the Tile-emitted program.
"""

import os
import sys

for _p in ("/opt/trn_rl_repo", "/root/.axon_site/_ro/trn_rl_repo"):
    if os.path.isdir(_p) and _p not in sys.path:
        sys.path.insert(0, _p)

import numpy as np
import ml_dtypes

import concourse.bass as bass
import concourse.tile as tile
from concourse import mybir
from concourse.bass_utils import run_bass_kernel_spmd
from concourse.masks import make_identity

NCORES = 8
B, M, JX, JQ, D = 32, 8, 512, 64, 128
BLOC = B // NCORES          # batches per core
NT = JX // 128              # 128-col i-blocks per (b,m)
NTH = NT // 2               # paired-row DMA t-blocks
F32 = mybir.dt.float32
BF16 = mybir.dt.bfloat16
FP16 = mybir.dt.float16


def _split_multi_waits(nc):
    """walrus encodes one sync-wait per instruction; Tile may attach several.
    Split the extras into standalone EventSemaphore (sequencer wait)
    instructions placed directly before the instruction on the same engine."""
    n = 0
    for fn in nc.m.functions:
        for bb in fn.blocks:
            out = []
            for inst in bb.instructions:
                si = inst.sync_info
                if si is not None and si.on_wait and len(si.on_wait) > 1:
                    waits = list(si.on_wait)
                    for k, w in enumerate(waits[:-1]):
                        out.append(mybir.InstEventSemaphore(
                            name=f"{inst.name}-sw{k}",
                            engine=inst.engine,
                            ins=[], outs=[],
                            sync_info=mybir.SyncInfo(on_wait=[w], on_update=[]),
                        ))
                        n += 1
                    inst.sync_info = mybir.SyncInfo(
                        on_wait=[waits[-1]], on_update=list(si.on_update))
                out.append(inst)
            bb.instructions = out
    return n


def _bcast(ap, reps, axis):
    """Stride-0 broadcast AP: insert [0, reps] at `axis` of ap's dims."""
    a = [list(d) for d in ap.ap]
    a.insert(axis, [0, reps])
    return bass.AP(tensor=ap.tensor, offset=ap.offset, ap=a)


def _col_bcast(ap_col, reps):
    """[128,1] column AP -> [128, reps] stride-0 stationary broadcast."""
    return bass.AP(tensor=ap_col.tensor, offset=ap_col.offset,
                   ap=[list(ap_col.ap[0]), [0, reps]])


CFG = dict(
    eng_textd="act",    # transposed-text PSUM->SBUF copy: act|dve
    eng_tabc="act",     # tabc normalize+cast: act|dve
    qa_merge=True,      # Z cols via separate tiny matmuls; single qa op
    col2_pool_u=1,      # how many of the 4 col2 u-blocks run on Pool
    col3_pool_u=4,      # how many of the 4 col3 u-blocks run on Pool
    q_tin="sync", q_out="sync", q_small="scalar",
    ptext=8, ptextd=3, pet=4, po123=8, psmall=12, ptabc=4,
    ttp=2, cross=1, etr=1, attnu=3, tabc=1,
    split_in=1, split_out=1, tail_split=4,
)


def _build_program():
    nc = bass.Bass()
    t_text = nc.dram_tensor("text", [BLOC, M, JX, D], BF16, kind="ExternalInput")
    # packed per-batch params: cols [0:65]=wq3aug [128 rows], [65:194]=qnaug
    # [rows 0:64 = [qn | ones]]
    t_pk = nc.dram_tensor("packed", [BLOC, D, D + JQ + 3], BF16, kind="ExternalInput")
    t_q2 = nc.dram_tensor("q2aug", [BLOC, JQ + 1, 1], F32, kind="ExternalInput")
    t_out = nc.dram_tensor("out", [BLOC, M, JX, 3 * D], FP16, kind="ExternalOutput")

    with tile.TileContext(nc) as tc:
        import contextlib
        ctx = contextlib.ExitStack()
        with ctx:
            singles = ctx.enter_context(tc.tile_pool(name="singles", bufs=1))
            perb = ctx.enter_context(tc.tile_pool(name="perb", bufs=2))
            ptext = ctx.enter_context(tc.tile_pool(name="ptext", bufs=CFG["ptext"]))
            ptextd = ctx.enter_context(tc.tile_pool(name="ptextd", bufs=CFG["ptextd"]))
            pet = ctx.enter_context(tc.tile_pool(name="pet", bufs=CFG["pet"]))
            po123 = ctx.enter_context(tc.tile_pool(name="po123", bufs=CFG["po123"]))
            psmall = ctx.enter_context(tc.tile_pool(name="psmall", bufs=CFG["psmall"]))
            ptabc = ctx.enter_context(tc.tile_pool(name="ptabc", bufs=CFG["ptabc"]))
            ps_ttp = ctx.enter_context(tc.tile_pool(name="ps_ttp", bufs=CFG["ttp"], space="PSUM"))
            ps_cross = ctx.enter_context(tc.tile_pool(name="ps_cross", bufs=CFG["cross"], space="PSUM"))
            ps_etr = ctx.enter_context(tc.tile_pool(name="ps_etr", bufs=CFG["etr"], space="PSUM"))
            ps_tabc = ctx.enter_context(tc.tile_pool(name="ps_tabc", bufs=CFG["tabc"], space="PSUM"))
            ps_attnu = ctx.enter_context(tc.tile_pool(name="ps_attnu", bufs=CFG["attnu"], space="PSUM"))

            # issue the very first text load before any constant setup so the
            # DMA engines start immediately
            first_text = ptext.tile([128, NT * D], BF16, tag="text")
            _fsrc = t_text[0, 0].rearrange("(t p k) d -> p t k d", p=128, k=2)
            getattr(nc, CFG["q_tin"]).dma_start(
                out=first_text.rearrange("p (t k d) -> p t k d", t=NTH, k=2),
                in_=_fsrc)

            identb = singles.tile([128, 128], BF16)
            make_identity(nc, identb)
            identb65 = singles.tile([JQ + 1, JQ + 1], BF16)
            make_identity(nc, identb65)
            ones128 = singles.tile([128, 128], BF16)
            nc.vector.memset(ones128, 1.0)

            def prep_text(gb, m):
                """Load + transpose + PSUM->SBUF copy of one text unit.
                Issued one unit AHEAD of the consuming compute so the ACT
                textd copy does not sit between exp(n) and cross(n+1) on the
                in-order ACT queue (that cycle gates the whole pipeline)."""
                # partition p, block u=2t+k holds DRAM row i=256t+2p+k
                if gb == 0 and m == 0:
                    text_il = first_text
                else:
                    text_il = ptext.tile([128, NT * D], BF16, tag="text")
                    src = t_text[gb, m].rearrange(
                        "(t p k) d -> p t k d", p=128, k=2)
                    dst = text_il.rearrange(
                        "p (t k d) -> p t k d", t=NTH, k=2)
                    nsi = CFG["split_in"]
                    for h in range(nsi):
                        hh = NTH // nsi
                        getattr(nc, CFG["q_tin"]).dma_start(
                            out=dst[:, h * hh:(h + 1) * hh],
                            in_=src[:, h * hh:(h + 1) * hh])
                ttp = ps_ttp.tile([128, JX], BF16, tag="ttp")
                for u in range(NT):
                    nc.tensor.transpose(
                        ttp[:, u * 128:(u + 1) * 128],
                        text_il[:, u * D:(u + 1) * D], identb)
                textd = ptextd.tile([128, JX], BF16, tag="textd")
                if CFG["eng_textd"] == "act":
                    nc.scalar.copy(out=textd, in_=ttp)
                else:
                    nc.vector.tensor_copy(textd, ttp)
                return text_il, textd

            pending = prep_text(0, 0)
            for gb in range(BLOC):
                pk_sb = perb.tile([D, D + JQ + 3], BF16, tag="pk")
                q2_sb = perb.tile([JQ + 1, 1], F32, tag="q2")
                qd = getattr(nc, CFG["q_small"])
                qd.dma_start(out=pk_sb, in_=t_pk[gb])
                qd.dma_start(out=q2_sb, in_=t_q2[gb])
                wq3_sb = pk_sb[:, 0:JQ + 1]
                qn_sb = pk_sb[0:JQ, JQ + 1: JQ + 1 + D + 1]

                for m in range(M):
                    unit = gb * M + m
                    text_il, textd = pending
                    if unit + 1 < BLOC * M:
                        pending = prep_text((unit + 1) // M, (unit + 1) % M)
                    text3 = text_il.rearrange("p (u d) -> p u d", d=D)

                    # ---- crossT_aug = [w3q|w1].T @ text_d  [65, 512] ----
                    cross = ps_cross.tile([JQ + 1, JX], F32, tag="cross")
                    nc.tensor.matmul(cross, wq3_sb, textd, start=True, stop=True)

                    # ---- eT = exp(cross + q2) (row 64 = exp(t1)) ----
                    eT = pet.tile([JQ + 1, JX], BF16, tag="eT")
                    nc.scalar.activation(
                        out=eT, in_=cross,
                        func=mybir.ActivationFunctionType.Exp,
                        bias=q2_sb[:, 0:1], scale=1.0)

                    # ---- transpose eT slices -> etr [128, 4*65] ----
                    etr = ps_etr.tile([128, NT * (JQ + 1)], BF16, tag="etr")
                    for u in range(NT):
                        nc.tensor.transpose(
                            etr[:, u * (JQ + 1):(u + 1) * (JQ + 1)],
                            eT[:, u * 128:(u + 1) * 128], identb65)
                    etr_blk = etr[:, :].rearrange("p (u j) -> p u j", j=JQ + 1)

                    # ---- qlmax path: etq = exp(qlmax) = G * exp(t1) ----
                    gq = psmall.tile([128, NT], BF16, tag="gq")
                    nc.vector.tensor_reduce(
                        out=gq, in_=etr_blk[:, :, 0:JQ],
                        axis=mybir.AxisListType.X, op=mybir.AluOpType.max)
                    etq = psmall.tile([128, NT], BF16, tag="etq")
                    nc.vector.tensor_mul(etq, gq, etr_blk[:, :, JQ])
                    etqs = psmall.tile([128, 1], BF16, tag="etqs")
                    with nc.allow_low_precision(
                            reason="4-element add of same-sign bf16"):
                        nc.vector.tensor_reduce(
                            out=etqs, in_=etq, axis=mybir.AxisListType.X,
                            op=mybir.AluOpType.add)

                    # ---- text_attn broadcast: every PE column = etq ----
                    # tabu regions: [0:D] text_attn bcast, [D] Zt bcast,
                    # [D+1:D+1+NT] attnu softmax denominators Z(i)
                    tabu = ps_tabc.tile([128, D + 1 + NT], F32, tag="tabu")
                    for u in range(NT):
                        nc.tensor.matmul(
                            tabu[:, 0:D],
                            _col_bcast(etq[:, u:u + 1], 128),
                            text_il[:, u * D:(u + 1) * D],
                            start=(u == 0), stop=(u == NT - 1))
                    # Zt on every partition (ones128 columns x etqs)
                    nc.tensor.matmul(tabu[:, D:D + 1], ones128, etqs,
                                     start=True, stop=True)
                    rzt = psmall.tile([128, 1], F32, tag="rzt")
                    nc.vector.reciprocal(out=rzt, in_=tabu[:, D:D + 1])
                    tabc = ptabc.tile([128, D], BF16, tag="tabc")
                    if CFG["eng_tabc"] == "act":
                        nc.scalar.mul(out=tabc, in_=tabu[:, 0:D], mul=rzt)
                    else:
                        nc.vector.tensor_scalar_mul(
                            out=tabc, in0=tabu[:, 0:D], scalar1=rzt)

                    # ---- attnu = eT[0:64].T @ qn; qa = attnu*rq ----
                    o123 = po123.tile([128, NT, 3 * D], FP16, tag="o123")
                    if CFG["qa_merge"]:
                        onesq = pk_sb[0:JQ, JQ + 1 + D:JQ + 2 + D]
                        attnu = ps_attnu.tile([128, NT * D], F32, tag="attnu")
                        for u in range(NT):
                            nc.tensor.matmul(
                                attnu[:, u * D:(u + 1) * D],
                                eT[0:JQ, u * 128:(u + 1) * 128],
                                qn_sb[:, 0:D], start=True, stop=True)
                        for u in range(NT):
                            nc.tensor.matmul(
                                tabu[:, D + 1 + u:D + 2 + u],
                                eT[0:JQ, u * 128:(u + 1) * 128],
                                onesq, start=True, stop=True)
                        rq = psmall.tile([128, NT], F32, tag="rq")
                        nc.vector.reciprocal(
                            out=rq, in_=tabu[:, D + 1:D + 1 + NT])
                        nc.vector.tensor_tensor(
                            out=o123[:, :, 0:D],
                            in0=attnu.rearrange("p (u d) -> p u d", d=D),
                            in1=_bcast(rq[:, :], D, 2),
                            op=mybir.AluOpType.mult)
                    else:
                        for h in range(2):
                            attnu = ps_attnu.tile([128, 2 * (D + 1)], F32,
                                                  tag="attnu")
                            a3 = attnu.rearrange("p (uu c) -> p uu c", c=D + 1)
                            for uu in range(2):
                                u = 2 * h + uu
                                nc.tensor.matmul(
                                    a3[:, uu, :],
                                    eT[0:JQ, u * 128:(u + 1) * 128],
                                    qn_sb, start=True, stop=True)
                            rq = psmall.tile([128, 2], F32, tag="rq")
                            nc.vector.reciprocal(out=rq, in_=a3[:, :, D])
                            nc.vector.tensor_tensor(
                                out=o123[:, 2 * h:2 * h + 2, 0:D],
                                in0=a3[:, :, 0:D],
                                in1=_bcast(rq[:, :], D, 2),
                                op=mybir.AluOpType.mult)

                    # ---- col2 = text*qa, col3 = text*text_attn; store ----
                    for (cl, cu), dve_u in (
                            ((D, 2 * D), NT - CFG["col2_pool_u"]),
                            ((2 * D, 3 * D), NT - CFG["col3_pool_u"])):
                        for eng, u0, u1 in ((nc.vector, 0, dve_u),
                                            (nc.gpsimd, dve_u, NT)):
                            if u1 <= u0:
                                continue
                            in1 = (o123[:, u0:u1, 0:D] if cl == D else
                                   _bcast(tabc[:, :], u1 - u0, 1))
                            eng.tensor_mul(
                                o123[:, u0:u1, cl:cu],
                                text3[:, u0:u1, :], in1)
                    nsp = CFG["split_out"]
                    if BLOC * M - unit <= CFG["tail_split"]:
                        nsp = max(nsp, 2)
                    ht = NT // nsp
                    dst4 = t_out[gb, m].rearrange(
                        "(t p k) c -> p t k c", p=128, k=2)
                    o1234 = o123[:, :, :].rearrange(
                        "p (t k) c -> p t k c", k=2)
                    for h in range(nsp):
                        ts0, ts1 = h * ht, (h + 1) * ht
                        getattr(nc, CFG["q_out"]).dma_start(
                            out=dst4[:, ts0 // 2:ts1 // 2],
                            in_=o1234[:, ts0 // 2:ts1 // 2])

    _split_multi_waits(nc)
    return nc


_NC_CACHE = {}


def _get_nc():
    if "nc" not in _NC_CACHE:
        _NC_CACHE["nc"] = _build_program()
    return _NC_CACHE["nc"]


def _make_in_maps(text, query, w):
    w1, w2, w3 = w[:D], w[D:2 * D], w[2 * D:]
    in_maps = []
    for c in range(NCORES):
        sl = slice(c * BLOC, (c + 1) * BLOC)
        q = query[sl]                                    # [BLOC, 64, 128]
        q2 = np.concatenate(
            [np.einsum("bjd,d->bj", q, w2),
             np.zeros((BLOC, 1), np.float32)], axis=1)[:, :, None]
        # packed [D, 65 + 129 + 1]: [0:65]=wq3aug; rows 0:64 of [65:194] =
        # [qn | ones]; col 194 pad (keeps row length odd->even alignment)
        pk = np.zeros((BLOC, D, D + JQ + 3), np.float32)
        pk[:, :, 0:JQ] = np.einsum("bjd->bdj", q * w3[None, None, :])
        pk[:, :, JQ] = w1[None, :]
        pk[:, 0:JQ, JQ + 1:JQ + 1 + D] = q
        pk[:, 0:JQ, JQ + 1 + D] = 1.0
        m = {
            "text": np.ascontiguousarray(text[sl]).astype(ml_dtypes.bfloat16),
            "packed": np.ascontiguousarray(pk).astype(ml_dtypes.bfloat16),
            "q2aug": np.ascontiguousarray(q2, dtype=np.float32),
        }
        in_maps.append(m)
    return in_maps


def kernel(text, query, text_mask, query_mask, w, b, _want_results=False):
    text = np.asarray(text, dtype=np.float32)
    query = np.asarray(query, dtype=np.float32)
    w = np.asarray(w, dtype=np.float32)
    nc = _get_nc()
    in_maps = _make_in_maps(text, query, w)
    res = run_bass_kernel_spmd(nc, in_maps, core_ids=list(range(NCORES)))
    out = np.empty((B, M, JX, 4 * D), dtype=np.float32)
    out[..., 0:D] = text
    for c in range(NCORES):
        out[c * BLOC:(c + 1) * BLOC, ..., D:] = res.results[c]["out"]
    if _want_results:
        return out, res
    return out


# revision 21
# speedup vs baseline: 1.0329x; 1.0329x over previous
"""BiAttention (BiDAF-style) Trainium2 kernel — 8-core SPMD, memory-bound.

Contract: kernel(**inputs) takes the FULL tensors
  text [32,8,512,128] f32, query [32,64,128] f32, text_mask [32,8,512],
  query_mask [32,64], w [384], b [1]
and returns attn [32,8,512,512] f32, matching the reference

  w1,w2,w3 = w[:128], w[128:256], w[256:]
  logits[b,m,i,j] = text[b,m,i]·(w3*query[b,j]) + t1[b,m,i] + q2[b,j] + b
  p_q   = softmax_j logits      -> query_attn = p_q @ query
  qlmax = max_j logits          -> p_text = softmax_i qlmax
  text_attn = sum_i p_text*text
  out = concat([text, query_attn, text*query_attn, text*text_attn], -1)

The masks are all ones per the problem spec, so the (1-mask)*VERY_NEG term is
identically zero; bias b and the per-row t1 shift cancel inside softmax_j
(t1 is carried exactly via an extra lhsT column for the qlmax path).

v3 — HBM-traffic-minimized (16.9 MB/core vs 41.9 baseline):
  * output block 0 is a verbatim copy of the input `text`; the host fills it
    during unshard; the device stores only [query_attn, text*query_attn,
    text*text_attn] in fp16.
  * text is loaded bf16 with a paired-row interleave (partition p holds rows
    {256t+2p, 256t+2p+1}) keeping every DMA descriptor >= 512 B contiguous.
  * all matmuls bf16 (1 PE col/cycle).
  * qn carries an appended ones column so the attnu matmul also emits the
    softmax_j denominators Z(i) per-partition.
  * text_attn: stride-0-broadcast stationary (every PE column = etq) makes
    the weighted-sum matmul emit its result broadcast across all 128
    partitions; one fused DVE tensor_scalar normalizes+casts it to bf16.
  * elementwise work is split across DVE / ACT / Pool per CFG.

Sharding: batch B=32 data-parallel across 8 NeuronCores (BLOC=4 per core),
32 (b,m) units per core, no collectives.  Host precomputes O(query)-sized
helpers only (packed into one bf16 tile per batch + tiny f32 q2).

Toolchain note: walrus in this container encodes ONE sync-wait per
instruction; _split_multi_waits() legalizes the Tile-emitted program.
"""

import os
import sys

for _p in ("/opt/trn_rl_repo", "/root/.axon_site/_ro/trn_rl_repo"):
    if os.path.isdir(_p) and _p not in sys.path:
        sys.path.insert(0, _p)

import numpy as np
import ml_dtypes

import concourse.bass as bass
import concourse.tile as tile
from concourse import mybir
from concourse.bass_utils import run_bass_kernel_spmd
from concourse.masks import make_identity

NCORES = 8
B, M, JX, JQ, D = 32, 8, 512, 64, 128
BLOC = B // NCORES          # batches per core
NT = JX // 128              # 128-col i-blocks per (b,m)
NTH = NT // 2               # paired-row DMA t-blocks
F32 = mybir.dt.float32
BF16 = mybir.dt.bfloat16
FP16 = mybir.dt.float16


def _split_multi_waits(nc):
    """walrus encodes one sync-wait per instruction; Tile may attach several.
    Split the extras into standalone EventSemaphore (sequencer wait)
    instructions placed directly before the instruction on the same engine."""
    n = 0
    for fn in nc.m.functions:
        for bb in fn.blocks:
            out = []
            for inst in bb.instructions:
                si = inst.sync_info
                if si is not None and si.on_wait and len(si.on_wait) > 1:
                    waits = list(si.on_wait)
                    for k, w in enumerate(waits[:-1]):
                        out.append(mybir.InstEventSemaphore(
                            name=f"{inst.name}-sw{k}",
                            engine=inst.engine,
                            ins=[], outs=[],
                            sync_info=mybir.SyncInfo(on_wait=[w], on_update=[]),
                        ))
                        n += 1
                    inst.sync_info = mybir.SyncInfo(
                        on_wait=[waits[-1]], on_update=list(si.on_update))
                out.append(inst)
            bb.instructions = out
    return n


def _bcast(ap, reps, axis):
    """Stride-0 broadcast AP: insert [0, reps] at `axis` of ap's dims."""
    a = [list(d) for d in ap.ap]
    a.insert(axis, [0, reps])
    return bass.AP(tensor=ap.tensor, offset=ap.offset, ap=a)


def _col_bcast(ap_col, reps):
    """[128,1] column AP -> [128, reps] stride-0 stationary broadcast."""
    return bass.AP(tensor=ap_col.tensor, offset=ap_col.offset,
                   ap=[list(ap_col.ap[0]), [0, reps]])


CFG = dict(
    eng_textd="act",    # transposed-text PSUM->SBUF copy: act|dve
    eng_tabc="act",     # tabc normalize+cast: act|dve
    qa_merge=True,      # Z cols via separate tiny matmuls; single qa op
    col2_pool_u=1,      # how many of the 4 col2 u-blocks run on Pool
    col3_pool_u=4,      # how many of the 4 col3 u-blocks run on Pool
    q_tin="sync", q_out="sync", q_small="scalar",
    ptext=8, ptextd=3, pet=4, po123=8, psmall=12, ptabc=4,
    ttp=2, cross=1, etr=1, attnu=3, tabc=1,
    split_in=1, split_out=1, tail_split=4,
)


def _build_program():
    nc = bass.Bass()
    t_text = nc.dram_tensor("text", [BLOC, M, JX, D], BF16, kind="ExternalInput")
    # packed per-batch params: cols [0:65]=wq3aug [128 rows], [65:194]=qnaug
    # [rows 0:64 = [qn | ones]]
    t_pk = nc.dram_tensor("packed", [BLOC, D, D + JQ + 3], BF16, kind="ExternalInput")
    t_q2 = nc.dram_tensor("q2aug", [BLOC, JQ + 1, 1], F32, kind="ExternalInput")
    t_out = nc.dram_tensor("out", [BLOC, M, JX, 3 * D], FP16, kind="ExternalOutput")

    with tile.TileContext(nc) as tc:
        import contextlib
        ctx = contextlib.ExitStack()
        with ctx:
            singles = ctx.enter_context(tc.tile_pool(name="singles", bufs=1))
            perb = ctx.enter_context(tc.tile_pool(name="perb", bufs=2))
            ptext = ctx.enter_context(tc.tile_pool(name="ptext", bufs=CFG["ptext"]))
            ptextd = ctx.enter_context(tc.tile_pool(name="ptextd", bufs=CFG["ptextd"]))
            pet = ctx.enter_context(tc.tile_pool(name="pet", bufs=CFG["pet"]))
            po123 = ctx.enter_context(tc.tile_pool(name="po123", bufs=CFG["po123"]))
            psmall = ctx.enter_context(tc.tile_pool(name="psmall", bufs=CFG["psmall"]))
            ptabc = ctx.enter_context(tc.tile_pool(name="ptabc", bufs=CFG["ptabc"]))
            ps_ttp = ctx.enter_context(tc.tile_pool(name="ps_ttp", bufs=CFG["ttp"], space="PSUM"))
            ps_cross = ctx.enter_context(tc.tile_pool(name="ps_cross", bufs=CFG["cross"], space="PSUM"))
            ps_etr = ctx.enter_context(tc.tile_pool(name="ps_etr", bufs=CFG["etr"], space="PSUM"))
            ps_tabc = ctx.enter_context(tc.tile_pool(name="ps_tabc", bufs=CFG["tabc"], space="PSUM"))
            ps_attnu = ctx.enter_context(tc.tile_pool(name="ps_attnu", bufs=CFG["attnu"], space="PSUM"))

            # issue the very first text load before any constant setup so the
            # DMA engines start immediately
            first_text = ptext.tile([128, NT * D], BF16, tag="text")
            _fsrc = t_text[0, 0].rearrange("(t p k) d -> p t k d", p=128, k=2)
            getattr(nc, CFG["q_tin"]).dma_start(
                out=first_text.rearrange("p (t k d) -> p t k d", t=NTH, k=2),
                in_=_fsrc)

            identb = singles.tile([128, 128], BF16)
            make_identity(nc, identb)
            identb65 = singles.tile([JQ + 1, JQ + 1], BF16)
            make_identity(nc, identb65)
            ones128 = singles.tile([128, 128], BF16)
            nc.vector.memset(ones128, 1.0)

            def prep_text(gb, m):
                """Load + transpose + PSUM->SBUF copy of one text unit.
                Issued one unit AHEAD of the consuming compute so the ACT
                textd copy does not sit between exp(n) and cross(n+1) on the
                in-order ACT queue (that cycle gates the whole pipeline)."""
                # partition p, block u=2t+k holds DRAM row i=256t+2p+k
                if gb == 0 and m == 0:
                    text_il = first_text
                else:
                    text_il = ptext.tile([128, NT * D], BF16, tag="text")
                    src = t_text[gb, m].rearrange(
                        "(t p k) d -> p t k d", p=128, k=2)
                    dst = text_il.rearrange(
                        "p (t k d) -> p t k d", t=NTH, k=2)
                    nsi = CFG["split_in"]
                    for h in range(nsi):
                        hh = NTH // nsi
                        getattr(nc, CFG["q_tin"]).dma_start(
                            out=dst[:, h * hh:(h + 1) * hh],
                            in_=src[:, h * hh:(h + 1) * hh])
                ttp = ps_ttp.tile([128, JX], BF16, tag="ttp")
                for u in range(NT):
                    nc.tensor.transpose(
                        ttp[:, u * 128:(u + 1) * 128],
                        text_il[:, u * D:(u + 1) * D], identb)
                textd = ptextd.tile([128, JX], BF16, tag="textd")
                if CFG["eng_textd"] == "act":
                    nc.scalar.copy(out=textd, in_=ttp)
                else:
                    nc.vector.tensor_copy(textd, ttp)
                return text_il, textd

            pending = prep_text(0, 0)
            for gb in range(BLOC):
                pk_sb = perb.tile([D, D + JQ + 3], BF16, tag="pk")
                q2_sb = perb.tile([JQ + 1, 1], F32, tag="q2")
                qd = getattr(nc, CFG["q_small"])
                qd.dma_start(out=pk_sb, in_=t_pk[gb])
                qd.dma_start(out=q2_sb, in_=t_q2[gb])
                wq3_sb = pk_sb[:, 0:JQ + 1]
                qn_sb = pk_sb[0:JQ, JQ + 1: JQ + 1 + D + 1]

                for m in range(M):
                    unit = gb * M + m
                    text_il, textd = pending
                    text3 = text_il.rearrange("p (u d) -> p u d", d=D)

                    # ---- crossT_aug = [w3q|w1].T @ text_d  [65, 512] ----
                    cross = ps_cross.tile([JQ + 1, JX], F32, tag="cross")
                    nc.tensor.matmul(cross, wq3_sb, textd, start=True, stop=True)

                    # ---- eT = exp(cross + q2) (row 64 = exp(t1)) ----
                    eT = pet.tile([JQ + 1, JX], BF16, tag="eT")
                    nc.scalar.activation(
                        out=eT, in_=cross,
                        func=mybir.ActivationFunctionType.Exp,
                        bias=q2_sb[:, 0:1], scale=1.0)

                    # prep the NEXT unit now: its ACT copy lands after exp(n)
                    # and its PE transposes after cross(n), so neither blocks
                    # the cross->exp critical cycle
                    if unit + 1 < BLOC * M:
                        pending = prep_text((unit + 1) // M, (unit + 1) % M)

                    # ---- transpose eT slices -> etr [128, 4*65] ----
                    etr = ps_etr.tile([128, NT * (JQ + 1)], BF16, tag="etr")
                    for u in range(NT):
                        nc.tensor.transpose(
                            etr[:, u * (JQ + 1):(u + 1) * (JQ + 1)],
                            eT[:, u * 128:(u + 1) * 128], identb65)
                    etr_blk = etr[:, :].rearrange("p (u j) -> p u j", j=JQ + 1)

                    # ---- qlmax path: etq = exp(qlmax) = G * exp(t1) ----
                    gq = psmall.tile([128, NT], BF16, tag="gq")
                    nc.vector.tensor_reduce(
                        out=gq, in_=etr_blk[:, :, 0:JQ],
                        axis=mybir.AxisListType.X, op=mybir.AluOpType.max)
                    etq = psmall.tile([128, NT], BF16, tag="etq")
                    nc.vector.tensor_mul(etq, gq, etr_blk[:, :, JQ])
                    etqs = psmall.tile([128, 1], BF16, tag="etqs")
                    with nc.allow_low_precision(
                            reason="4-element add of same-sign bf16"):
                        nc.vector.tensor_reduce(
                            out=etqs, in_=etq, axis=mybir.AxisListType.X,
                            op=mybir.AluOpType.add)

                    # ---- text_attn broadcast: every PE column = etq ----
                    # tabu regions: [0:D] text_attn bcast, [D] Zt bcast,
                    # [D+1:D+1+NT] attnu softmax denominators Z(i)
                    tabu = ps_tabc.tile([128, D + 1 + NT], F32, tag="tabu")
                    for u in range(NT):
                        nc.tensor.matmul(
                            tabu[:, 0:D],
                            _col_bcast(etq[:, u:u + 1], 128),
                            text_il[:, u * D:(u + 1) * D],
                            start=(u == 0), stop=(u == NT - 1))
                    # Zt on every partition (ones128 columns x etqs)
                    nc.tensor.matmul(tabu[:, D:D + 1], ones128, etqs,
                                     start=True, stop=True)
                    rzt = psmall.tile([128, 1], F32, tag="rzt")
                    nc.vector.reciprocal(out=rzt, in_=tabu[:, D:D + 1])
                    tabc = ptabc.tile([128, D], BF16, tag="tabc")
                    if CFG["eng_tabc"] == "act":
                        nc.scalar.mul(out=tabc, in_=tabu[:, 0:D], mul=rzt)
                    else:
                        nc.vector.tensor_scalar_mul(
                            out=tabc, in0=tabu[:, 0:D], scalar1=rzt)

                    # ---- attnu = eT[0:64].T @ qn; qa = attnu*rq ----
                    o123 = po123.tile([128, NT, 3 * D], FP16, tag="o123")
                    if CFG["qa_merge"]:
                        onesq = pk_sb[0:JQ, JQ + 1 + D:JQ + 2 + D]
                        attnu = ps_attnu.tile([128, NT * D], F32, tag="attnu")
                        for u in range(NT):
                            nc.tensor.matmul(
                                attnu[:, u * D:(u + 1) * D],
                                eT[0:JQ, u * 128:(u + 1) * 128],
                                qn_sb[:, 0:D], start=True, stop=True)
                        for u in range(NT):
                            nc.tensor.matmul(
                                tabu[:, D + 1 + u:D + 2 + u],
                                eT[0:JQ, u * 128:(u + 1) * 128],
                                onesq, start=True, stop=True)
                        rq = psmall.tile([128, NT], F32, tag="rq")
                        nc.vector.reciprocal(
                            out=rq, in_=tabu[:, D + 1:D + 1 + NT])
                        nc.vector.tensor_tensor(
                            out=o123[:, :, 0:D],
                            in0=attnu.rearrange("p (u d) -> p u d", d=D),
                            in1=_bcast(rq[:, :], D, 2),
                            op=mybir.AluOpType.mult)
                    else:
                        for h in range(2):
                            attnu = ps_attnu.tile([128, 2 * (D + 1)], F32,
                                                  tag="attnu")
                            a3 = attnu.rearrange("p (uu c) -> p uu c", c=D + 1)
                            for uu in range(2):
                                u = 2 * h + uu
                                nc.tensor.matmul(
                                    a3[:, uu, :],
                                    eT[0:JQ, u * 128:(u + 1) * 128],
                                    qn_sb, start=True, stop=True)
                            rq = psmall.tile([128, 2], F32, tag="rq")
                            nc.vector.reciprocal(out=rq, in_=a3[:, :, D])
                            nc.vector.tensor_tensor(
                                out=o123[:, 2 * h:2 * h + 2, 0:D],
                                in0=a3[:, :, 0:D],
                                in1=_bcast(rq[:, :], D, 2),
                                op=mybir.AluOpType.mult)

                    # ---- col2 = text*qa, col3 = text*text_attn; store ----
                    for (cl, cu), dve_u in (
                            ((D, 2 * D), NT - CFG["col2_pool_u"]),
                            ((2 * D, 3 * D), NT - CFG["col3_pool_u"])):
                        for eng, u0, u1 in ((nc.vector, 0, dve_u),
                                            (nc.gpsimd, dve_u, NT)):
                            if u1 <= u0:
                                continue
                            in1 = (o123[:, u0:u1, 0:D] if cl == D else
                                   _bcast(tabc[:, :], u1 - u0, 1))
                            eng.tensor_mul(
                                o123[:, u0:u1, cl:cu],
                                text3[:, u0:u1, :], in1)
                    nsp = CFG["split_out"]
                    if BLOC * M - unit <= CFG["tail_split"]:
                        nsp = max(nsp, 2)
                    ht = NT // nsp
                    dst4 = t_out[gb, m].rearrange(
                        "(t p k) c -> p t k c", p=128, k=2)
                    o1234 = o123[:, :, :].rearrange(
                        "p (t k) c -> p t k c", k=2)
                    for h in range(nsp):
                        ts0, ts1 = h * ht, (h + 1) * ht
                        getattr(nc, CFG["q_out"]).dma_start(
                            out=dst4[:, ts0 // 2:ts1 // 2],
                            in_=o1234[:, ts0 // 2:ts1 // 2])

    _split_multi_waits(nc)
    return nc


_NC_CACHE = {}


def _get_nc():
    if "nc" not in _NC_CACHE:
        _NC_CACHE["nc"] = _build_program()
    return _NC_CACHE["nc"]


def _make_in_maps(text, query, w):
    w1, w2, w3 = w[:D], w[D:2 * D], w[2 * D:]
    in_maps = []
    for c in range(NCORES):
        sl = slice(c * BLOC, (c + 1) * BLOC)
        q = query[sl]                                    # [BLOC, 64, 128]
        q2 = np.concatenate(
            [np.einsum("bjd,d->bj", q, w2),
             np.zeros((BLOC, 1), np.float32)], axis=1)[:, :, None]
        # packed [D, 65 + 129 + 1]: [0:65]=wq3aug; rows 0:64 of [65:194] =
        # [qn | ones]; col 194 pad (keeps row length odd->even alignment)
        pk = np.zeros((BLOC, D, D + JQ + 3), np.float32)
        pk[:, :, 0:JQ] = np.einsum("bjd->bdj", q * w3[None, None, :])
        pk[:, :, JQ] = w1[None, :]
        pk[:, 0:JQ, JQ + 1:JQ + 1 + D] = q
        pk[:, 0:JQ, JQ + 1 + D] = 1.0
        m = {
            "text": np.ascontiguousarray(text[sl]).astype(ml_dtypes.bfloat16),
            "packed": np.ascontiguousarray(pk).astype(ml_dtypes.bfloat16),
            "q2aug": np.ascontiguousarray(q2, dtype=np.float32),
        }
        in_maps.append(m)
    return in_maps


def kernel(text, query, text_mask, query_mask, w, b, _want_results=False):
    text = np.asarray(text, dtype=np.float32)
    query = np.asarray(query, dtype=np.float32)
    w = np.asarray(w, dtype=np.float32)
    nc = _get_nc()
    in_maps = _make_in_maps(text, query, w)
    res = run_bass_kernel_spmd(nc, in_maps, core_ids=list(range(NCORES)))
    out = np.empty((B, M, JX, 4 * D), dtype=np.float32)
    out[..., 0:D] = text
    for c in range(NCORES):
        out[c * BLOC:(c + 1) * BLOC, ..., D:] = res.results[c]["out"]
    if _want_results:
        return out, res
    return out


# revision 27
# speedup vs baseline: 1.0540x; 1.0204x over previous
"""BiAttention (BiDAF-style) Trainium2 kernel — 8-core SPMD, memory-bound.

Contract: kernel(**inputs) takes the FULL tensors
  text [32,8,512,128] f32, query [32,64,128] f32, text_mask [32,8,512],
  query_mask [32,64], w [384], b [1]
and returns attn [32,8,512,512] f32, matching the reference

  w1,w2,w3 = w[:128], w[128:256], w[256:]
  logits[b,m,i,j] = text[b,m,i]·(w3*query[b,j]) + t1[b,m,i] + q2[b,j] + b
  p_q   = softmax_j logits      -> query_attn = p_q @ query
  qlmax = max_j logits          -> p_text = softmax_i qlmax
  text_attn = sum_i p_text*text
  out = concat([text, query_attn, text*query_attn, text*text_attn], -1)

The masks are all ones per the problem spec, so the (1-mask)*VERY_NEG term is
identically zero; bias b and the per-row t1 shift cancel inside softmax_j
(t1 is carried exactly via an extra lhsT column for the qlmax path).

v3 — HBM-traffic-minimized (16.9 MB/core vs 41.9 baseline):
  * output block 0 is a verbatim copy of the input `text`; the host fills it
    during unshard; the device stores only [query_attn, text*query_attn,
    text*text_attn] in fp16.
  * text is loaded bf16 with a paired-row interleave (partition p holds rows
    {256t+2p, 256t+2p+1}) keeping every DMA descriptor >= 512 B contiguous.
  * all matmuls bf16 (1 PE col/cycle).
  * qn carries an appended ones column so the attnu matmul also emits the
    softmax_j denominators Z(i) per-partition.
  * text_attn: stride-0-broadcast stationary (every PE column = etq) makes
    the weighted-sum matmul emit its result broadcast across all 128
    partitions; one fused DVE tensor_scalar normalizes+casts it to bf16.
  * elementwise work is split across DVE / ACT / Pool per CFG.

Sharding: batch B=32 data-parallel across 8 NeuronCores (BLOC=4 per core),
32 (b,m) units per core, no collectives.  Host precomputes O(query)-sized
helpers only (packed into one bf16 tile per batch + tiny f32 q2).

Toolchain note: walrus in this container encodes ONE sync-wait per
instruction; _split_multi_waits() legalizes the Tile-emitted program.
"""

import os
import sys

for _p in ("/opt/trn_rl_repo", "/root/.axon_site/_ro/trn_rl_repo"):
    if os.path.isdir(_p) and _p not in sys.path:
        sys.path.insert(0, _p)

import numpy as np
import ml_dtypes

import concourse.bass as bass
import concourse.tile as tile
from concourse import mybir
from concourse.bass_utils import run_bass_kernel_spmd
from concourse.masks import make_identity

NCORES = 8
B, M, JX, JQ, D = 32, 8, 512, 64, 128
BLOC = B // NCORES          # batches per core
NT = JX // 128              # 128-col i-blocks per (b,m)
NTH = NT // 2               # paired-row DMA t-blocks
F32 = mybir.dt.float32
BF16 = mybir.dt.bfloat16
FP16 = mybir.dt.float16


def _split_multi_waits(nc):
    """walrus encodes one sync-wait per instruction; Tile may attach several.
    Split the extras into standalone EventSemaphore (sequencer wait)
    instructions placed directly before the instruction on the same engine."""
    n = 0
    for fn in nc.m.functions:
        for bb in fn.blocks:
            out = []
            for inst in bb.instructions:
                si = inst.sync_info
                if si is not None and si.on_wait and len(si.on_wait) > 1:
                    waits = list(si.on_wait)
                    for k, w in enumerate(waits[:-1]):
                        out.append(mybir.InstEventSemaphore(
                            name=f"{inst.name}-sw{k}",
                            engine=inst.engine,
                            ins=[], outs=[],
                            sync_info=mybir.SyncInfo(on_wait=[w], on_update=[]),
                        ))
                        n += 1
                    inst.sync_info = mybir.SyncInfo(
                        on_wait=[waits[-1]], on_update=list(si.on_update))
                out.append(inst)
            bb.instructions = out
    return n


def _bcast(ap, reps, axis):
    """Stride-0 broadcast AP: insert [0, reps] at `axis` of ap's dims."""
    a = [list(d) for d in ap.ap]
    a.insert(axis, [0, reps])
    return bass.AP(tensor=ap.tensor, offset=ap.offset, ap=a)


def _col_bcast(ap_col, reps):
    """[128,1] column AP -> [128, reps] stride-0 stationary broadcast."""
    return bass.AP(tensor=ap_col.tensor, offset=ap_col.offset,
                   ap=[list(ap_col.ap[0]), [0, reps]])


CFG = dict(
    eng_textd="act",    # transposed-text PSUM->SBUF copy: act|dve
    eng_tabc="act",     # tabc normalize+cast: act|dve
    qa_merge=True,      # Z cols via separate tiny matmuls; single qa op
    col2_pool_u=2,      # how many of the 4 col2 u-blocks run on Pool
    col3_pool_u=4,      # how many of the 4 col3 u-blocks run on Pool
    q_tin="sync", q_out="sync", q_small="scalar",
    ptext=12, ptextd=3, pet=4, po123=12, psmall=12, ptabc=4,
    ttp=2, cross=1, etr=1, attnu=3, tabc=1,
    split_in=1, split_out=1, tail_split=4,
)


def _build_program():
    nc = bass.Bass()
    t_text = nc.dram_tensor("text", [BLOC, M, JX, D], BF16, kind="ExternalInput")
    # packed per-batch params: cols [0:65]=wq3aug [128 rows], [65:194]=qnaug
    # [rows 0:64 = [qn | ones]]
    t_pk = nc.dram_tensor("packed", [BLOC, D, D + JQ + 3], BF16, kind="ExternalInput")
    t_q2 = nc.dram_tensor("q2aug", [BLOC, JQ + 1, 1], F32, kind="ExternalInput")
    t_out = nc.dram_tensor("out", [BLOC, M, JX, 3 * D], FP16, kind="ExternalOutput")

    with tile.TileContext(nc) as tc:
        import contextlib
        ctx = contextlib.ExitStack()
        with ctx:
            singles = ctx.enter_context(tc.tile_pool(name="singles", bufs=1))
            perb = ctx.enter_context(tc.tile_pool(name="perb", bufs=2))
            ptext = ctx.enter_context(tc.tile_pool(name="ptext", bufs=CFG["ptext"]))
            ptextd = ctx.enter_context(tc.tile_pool(name="ptextd", bufs=CFG["ptextd"]))
            pet = ctx.enter_context(tc.tile_pool(name="pet", bufs=CFG["pet"]))
            po123 = ctx.enter_context(tc.tile_pool(name="po123", bufs=CFG["po123"]))
            psmall = ctx.enter_context(tc.tile_pool(name="psmall", bufs=CFG["psmall"]))
            ptabc = ctx.enter_context(tc.tile_pool(name="ptabc", bufs=CFG["ptabc"]))
            ps_ttp = ctx.enter_context(tc.tile_pool(name="ps_ttp", bufs=CFG["ttp"], space="PSUM"))
            ps_cross = ctx.enter_context(tc.tile_pool(name="ps_cross", bufs=CFG["cross"], space="PSUM"))
            ps_etr = ctx.enter_context(tc.tile_pool(name="ps_etr", bufs=CFG["etr"], space="PSUM"))
            ps_tabc = ctx.enter_context(tc.tile_pool(name="ps_tabc", bufs=CFG["tabc"], space="PSUM"))
            ps_attnu = ctx.enter_context(tc.tile_pool(name="ps_attnu", bufs=CFG["attnu"], space="PSUM"))

            # issue the very first text load before any constant setup so the
            # DMA engines start immediately
            first_text = ptext.tile([128, NT * D], BF16, tag="text")
            _fsrc = t_text[0, 0].rearrange("(t p k) d -> p t k d", p=128, k=2)
            getattr(nc, CFG["q_tin"]).dma_start(
                out=first_text.rearrange("p (t k d) -> p t k d", t=NTH, k=2),
                in_=_fsrc)

            identb = singles.tile([128, 128], BF16)
            make_identity(nc, identb)
            identb65 = singles.tile([JQ + 1, JQ + 1], BF16)
            make_identity(nc, identb65)
            ones128 = singles.tile([128, 128], BF16)
            nc.vector.memset(ones128, 1.0)

            def prep_text(gb, m):
                """Load + transpose + PSUM->SBUF copy of one text unit.
                Issued one unit AHEAD of the consuming compute so the ACT
                textd copy does not sit between exp(n) and cross(n+1) on the
                in-order ACT queue (that cycle gates the whole pipeline)."""
                # partition p, block u=2t+k holds DRAM row i=256t+2p+k
                if gb == 0 and m == 0:
                    text_il = first_text
                else:
                    text_il = ptext.tile([128, NT * D], BF16, tag="text")
                    src = t_text[gb, m].rearrange(
                        "(t p k) d -> p t k d", p=128, k=2)
                    dst = text_il.rearrange(
                        "p (t k d) -> p t k d", t=NTH, k=2)
                    nsi = CFG["split_in"]
                    for h in range(nsi):
                        hh = NTH // nsi
                        getattr(nc, CFG["q_tin"]).dma_start(
                            out=dst[:, h * hh:(h + 1) * hh],
                            in_=src[:, h * hh:(h + 1) * hh])
                ttp = ps_ttp.tile([128, JX], BF16, tag="ttp")
                for u in range(NT):
                    nc.tensor.transpose(
                        ttp[:, u * 128:(u + 1) * 128],
                        text_il[:, u * D:(u + 1) * D], identb)
                textd = ptextd.tile([128, JX], BF16, tag="textd")
                if CFG["eng_textd"] == "act":
                    nc.scalar.copy(out=textd, in_=ttp)
                else:
                    nc.vector.tensor_copy(textd, ttp)
                return text_il, textd

            pending = prep_text(0, 0)
            for gb in range(BLOC):
                pk_sb = perb.tile([D, D + JQ + 3], BF16, tag="pk")
                q2_sb = perb.tile([JQ + 1, 1], F32, tag="q2")
                qd = getattr(nc, CFG["q_small"])
                qd.dma_start(out=pk_sb, in_=t_pk[gb])
                qd.dma_start(out=q2_sb, in_=t_q2[gb])
                wq3_sb = pk_sb[:, 0:JQ + 1]
                qn_sb = pk_sb[0:JQ, JQ + 1: JQ + 1 + D + 1]

                for m in range(M):
                    unit = gb * M + m
                    text_il, textd = pending
                    text3 = text_il.rearrange("p (u d) -> p u d", d=D)

                    # ---- crossT_aug = [w3q|w1].T @ text_d  [65, 512] ----
                    cross = ps_cross.tile([JQ + 1, JX], F32, tag="cross")
                    nc.tensor.matmul(cross, wq3_sb, textd, start=True, stop=True)

                    # ---- eT = exp(cross + q2) (row 64 = exp(t1)) ----
                    eT = pet.tile([JQ + 1, JX], BF16, tag="eT")
                    nc.scalar.activation(
                        out=eT, in_=cross,
                        func=mybir.ActivationFunctionType.Exp,
                        bias=q2_sb[:, 0:1], scale=1.0)

                    # prep the NEXT unit now: its ACT copy lands after exp(n)
                    # and its PE transposes after cross(n), so neither blocks
                    # the cross->exp critical cycle
                    if unit + 1 < BLOC * M:
                        pending = prep_text((unit + 1) // M, (unit + 1) % M)

                    # ---- transpose eT slices -> etr [128, 4*65] ----
                    etr = ps_etr.tile([128, NT * (JQ + 1)], BF16, tag="etr")
                    for u in range(NT):
                        nc.tensor.transpose(
                            etr[:, u * (JQ + 1):(u + 1) * (JQ + 1)],
                            eT[:, u * 128:(u + 1) * 128], identb65)
                    etr_blk = etr[:, :].rearrange("p (u j) -> p u j", j=JQ + 1)

                    # ---- qlmax path: etq = exp(qlmax) = G * exp(t1) ----
                    gq = psmall.tile([128, NT], BF16, tag="gq")
                    nc.vector.tensor_reduce(
                        out=gq, in_=etr_blk[:, :, 0:JQ],
                        axis=mybir.AxisListType.X, op=mybir.AluOpType.max)
                    etq = psmall.tile([128, NT], BF16, tag="etq")
                    nc.vector.tensor_mul(etq, gq, etr_blk[:, :, JQ])

                    # ---- attnu = eT[0:64].T @ qn; qa = attnu*rq ----
                    # tabu regions: [0:D] text_attn bcast, [D] Zt bcast,
                    # [D+1:D+1+NT] attnu softmax denominators Z(i)
                    tabu = ps_tabc.tile([128, D + 1 + NT], F32, tag="tabu")
                    o123 = po123.tile([128, NT, 3 * D], FP16, tag="o123")
                    if CFG["qa_merge"]:
                        onesq = pk_sb[0:JQ, JQ + 1 + D:JQ + 2 + D]
                        attnu = ps_attnu.tile([128, NT * D], F32, tag="attnu")
                        for u in range(NT):
                            nc.tensor.matmul(
                                attnu[:, u * D:(u + 1) * D],
                                eT[0:JQ, u * 128:(u + 1) * 128],
                                qn_sb[:, 0:D], start=True, stop=True)
                        for u in range(NT):
                            nc.tensor.matmul(
                                tabu[:, D + 1 + u:D + 2 + u],
                                eT[0:JQ, u * 128:(u + 1) * 128],
                                onesq, start=True, stop=True)
                        rq = psmall.tile([128, NT], F32, tag="rq")
                        nc.vector.reciprocal(
                            out=rq, in_=tabu[:, D + 1:D + 1 + NT])
                        nc.vector.tensor_tensor(
                            out=o123[:, :, 0:D],
                            in0=attnu.rearrange("p (u d) -> p u d", d=D),
                            in1=_bcast(rq[:, :], D, 2),
                            op=mybir.AluOpType.mult)
                    else:
                        for h in range(2):
                            attnu = ps_attnu.tile([128, 2 * (D + 1)], F32,
                                                  tag="attnu")
                            a3 = attnu.rearrange("p (uu c) -> p uu c", c=D + 1)
                            for uu in range(2):
                                u = 2 * h + uu
                                nc.tensor.matmul(
                                    a3[:, uu, :],
                                    eT[0:JQ, u * 128:(u + 1) * 128],
                                    qn_sb, start=True, stop=True)
                            rq = psmall.tile([128, 2], F32, tag="rq")
                            nc.vector.reciprocal(out=rq, in_=a3[:, :, D])
                            nc.vector.tensor_tensor(
                                out=o123[:, 2 * h:2 * h + 2, 0:D],
                                in0=a3[:, :, 0:D],
                                in1=_bcast(rq[:, :], D, 2),
                                op=mybir.AluOpType.mult)

                    # ---- text_attn broadcast: every PE column = etq ----
                    for u in range(NT):
                        nc.tensor.matmul(
                            tabu[:, 0:D],
                            _col_bcast(etq[:, u:u + 1], 128),
                            text_il[:, u * D:(u + 1) * D],
                            start=(u == 0), stop=(u == NT - 1))
                    # Zt on every partition: sum_u sum_p etq[p,u], via 4
                    # accumulating 1-column matmuls (etq column broadcast
                    # as stationary, ones column moving)
                    for u in range(NT):
                        nc.tensor.matmul(
                            tabu[:, D:D + 1],
                            _col_bcast(etq[:, u:u + 1], 128),
                            ones128[:, 0:1],
                            start=(u == 0), stop=(u == NT - 1))
                    rzt = psmall.tile([128, 1], F32, tag="rzt")
                    nc.vector.reciprocal(out=rzt, in_=tabu[:, D:D + 1])
                    tabc = ptabc.tile([128, D], BF16, tag="tabc")
                    if CFG["eng_tabc"] == "act":
                        nc.scalar.mul(out=tabc, in_=tabu[:, 0:D], mul=rzt)
                    else:
                        nc.vector.tensor_scalar_mul(
                            out=tabc, in0=tabu[:, 0:D], scalar1=rzt)

                    # ---- col2 = text*qa, col3 = text*text_attn; store ----
                    for (cl, cu), dve_u in (
                            ((D, 2 * D), NT - CFG["col2_pool_u"]),
                            ((2 * D, 3 * D), NT - CFG["col3_pool_u"])):
                        for eng, u0, u1 in ((nc.vector, 0, dve_u),
                                            (nc.gpsimd, dve_u, NT)):
                            if u1 <= u0:
                                continue
                            in1 = (o123[:, u0:u1, 0:D] if cl == D else
                                   _bcast(tabc[:, :], u1 - u0, 1))
                            eng.tensor_mul(
                                o123[:, u0:u1, cl:cu],
                                text3[:, u0:u1, :], in1)
                    nsp = CFG["split_out"]
                    if BLOC * M - unit <= CFG["tail_split"]:
                        nsp = max(nsp, 2)
                    ht = NT // nsp
                    dst4 = t_out[gb, m].rearrange(
                        "(t p k) c -> p t k c", p=128, k=2)
                    o1234 = o123[:, :, :].rearrange(
                        "p (t k) c -> p t k c", k=2)
                    for h in range(nsp):
                        ts0, ts1 = h * ht, (h + 1) * ht
                        getattr(nc, CFG["q_out"]).dma_start(
                            out=dst4[:, ts0 // 2:ts1 // 2],
                            in_=o1234[:, ts0 // 2:ts1 // 2])

    _split_multi_waits(nc)
    return nc


_NC_CACHE = {}


def _get_nc():
    if "nc" not in _NC_CACHE:
        _NC_CACHE["nc"] = _build_program()
    return _NC_CACHE["nc"]


def _make_in_maps(text, query, w):
    w1, w2, w3 = w[:D], w[D:2 * D], w[2 * D:]
    in_maps = []
    for c in range(NCORES):
        sl = slice(c * BLOC, (c + 1) * BLOC)
        q = query[sl]                                    # [BLOC, 64, 128]
        q2 = np.concatenate(
            [np.einsum("bjd,d->bj", q, w2),
             np.zeros((BLOC, 1), np.float32)], axis=1)[:, :, None]
        # packed [D, 65 + 129 + 1]: [0:65]=wq3aug; rows 0:64 of [65:194] =
        # [qn | ones]; col 194 pad (keeps row length odd->even alignment)
        pk = np.zeros((BLOC, D, D + JQ + 3), np.float32)
        pk[:, :, 0:JQ] = np.einsum("bjd->bdj", q * w3[None, None, :])
        pk[:, :, JQ] = w1[None, :]
        pk[:, 0:JQ, JQ + 1:JQ + 1 + D] = q
        pk[:, 0:JQ, JQ + 1 + D] = 1.0
        m = {
            "text": np.ascontiguousarray(text[sl]).astype(ml_dtypes.bfloat16),
            "packed": np.ascontiguousarray(pk).astype(ml_dtypes.bfloat16),
            "q2aug": np.ascontiguousarray(q2, dtype=np.float32),
        }
        in_maps.append(m)
    return in_maps


def kernel(text, query, text_mask, query_mask, w, b, _want_results=False):
    text = np.asarray(text, dtype=np.float32)
    query = np.asarray(query, dtype=np.float32)
    w = np.asarray(w, dtype=np.float32)
    nc = _get_nc()
    in_maps = _make_in_maps(text, query, w)
    res = run_bass_kernel_spmd(nc, in_maps, core_ids=list(range(NCORES)))
    out = np.empty((B, M, JX, 4 * D), dtype=np.float32)
    out[..., 0:D] = text
    for c in range(NCORES):
        out[c * BLOC:(c + 1) * BLOC, ..., D:] = res.results[c]["out"]
    if _want_results:
        return out, res
    return out


# revision 31
# speedup vs baseline: 1.1011x; 1.0448x over previous
"""BiAttention (BiDAF-style) Trainium2 kernel — 8-core SPMD, memory-bound.

Contract: kernel(**inputs) takes the FULL tensors
  text [32,8,512,128] f32, query [32,64,128] f32, text_mask [32,8,512],
  query_mask [32,64], w [384], b [1]
and returns attn [32,8,512,512] f32, matching the reference

  w1,w2,w3 = w[:128], w[128:256], w[256:]
  logits[b,m,i,j] = text[b,m,i]·(w3*query[b,j]) + t1[b,m,i] + q2[b,j] + b
  p_q   = softmax_j logits      -> query_attn = p_q @ query
  qlmax = max_j logits          -> p_text = softmax_i qlmax
  text_attn = sum_i p_text*text
  out = concat([text, query_attn, text*query_attn, text*text_attn], -1)

The masks are all ones per the problem spec, so the (1-mask)*VERY_NEG term is
identically zero; bias b and the per-row t1 shift cancel inside softmax_j
(t1 is carried exactly via an extra lhsT column for the qlmax path).

v3 — HBM-traffic-minimized (16.9 MB/core vs 41.9 baseline):
  * output block 0 is a verbatim copy of the input `text`; the host fills it
    during unshard; the device stores only [query_attn, text*query_attn,
    text*text_attn] in fp16.
  * text is loaded bf16 with a paired-row interleave (partition p holds rows
    {256t+2p, 256t+2p+1}) keeping every DMA descriptor >= 512 B contiguous.
  * all matmuls bf16 (1 PE col/cycle).
  * qn carries an appended ones column so the attnu matmul also emits the
    softmax_j denominators Z(i) per-partition.
  * text_attn: stride-0-broadcast stationary (every PE column = etq) makes
    the weighted-sum matmul emit its result broadcast across all 128
    partitions; one fused DVE tensor_scalar normalizes+casts it to bf16.
  * elementwise work is split across DVE / ACT / Pool per CFG.

Sharding: batch B=32 data-parallel across 8 NeuronCores (BLOC=4 per core),
32 (b,m) units per core, no collectives.  Host precomputes O(query)-sized
helpers only (packed into one bf16 tile per batch + tiny f32 q2).

Toolchain note: walrus in this container encodes ONE sync-wait per
instruction; _split_multi_waits() legalizes the Tile-emitted program.
"""

import os
import sys

for _p in ("/opt/trn_rl_repo", "/root/.axon_site/_ro/trn_rl_repo"):
    if os.path.isdir(_p) and _p not in sys.path:
        sys.path.insert(0, _p)

import numpy as np
import ml_dtypes

import concourse.bass as bass
import concourse.tile as tile
from concourse import mybir
from concourse.bass_utils import run_bass_kernel_spmd
from concourse.masks import make_identity

NCORES = 8
B, M, JX, JQ, D = 32, 8, 512, 64, 128
BLOC = B // NCORES          # batches per core
NT = JX // 128              # 128-col i-blocks per (b,m)
NTH = NT // 2               # paired-row DMA t-blocks
F32 = mybir.dt.float32
BF16 = mybir.dt.bfloat16
FP16 = mybir.dt.float16


def _split_multi_waits(nc):
    """walrus encodes one sync-wait per instruction; Tile may attach several.
    Split the extras into standalone EventSemaphore (sequencer wait)
    instructions placed directly before the instruction on the same engine."""
    n = 0
    for fn in nc.m.functions:
        for bb in fn.blocks:
            out = []
            for inst in bb.instructions:
                si = inst.sync_info
                if si is not None and si.on_wait and len(si.on_wait) > 1:
                    waits = list(si.on_wait)
                    for k, w in enumerate(waits[:-1]):
                        out.append(mybir.InstEventSemaphore(
                            name=f"{inst.name}-sw{k}",
                            engine=inst.engine,
                            ins=[], outs=[],
                            sync_info=mybir.SyncInfo(on_wait=[w], on_update=[]),
                        ))
                        n += 1
                    inst.sync_info = mybir.SyncInfo(
                        on_wait=[waits[-1]], on_update=list(si.on_update))
                out.append(inst)
            bb.instructions = out
    return n


def _bcast(ap, reps, axis):
    """Stride-0 broadcast AP: insert [0, reps] at `axis` of ap's dims."""
    a = [list(d) for d in ap.ap]
    a.insert(axis, [0, reps])
    return bass.AP(tensor=ap.tensor, offset=ap.offset, ap=a)


def _col_bcast(ap_col, reps):
    """[128,1] column AP -> [128, reps] stride-0 stationary broadcast."""
    return bass.AP(tensor=ap_col.tensor, offset=ap_col.offset,
                   ap=[list(ap_col.ap[0]), [0, reps]])


CFG = dict(
    eng_textd="act",    # transposed-text PSUM->SBUF copy: act|dve
    eng_tabc="act",     # tabc normalize+cast: act|dve
    qa_merge=True,      # Z cols via separate tiny matmuls; single qa op
    col2_pool_u=2,      # how many of the 4 col2 u-blocks run on Pool
    col3_pool_u=4,      # how many of the 4 col3 u-blocks run on Pool
    q_tin="sync", q_out="sync", q_small="scalar",
    ptext=12, ptextd=3, pet=4, po123=12, psmall=12, ptabc=4,
    ttp=2, cross=1, etr=1, attnu=2, tabc=2,
    split_in=1, split_out=1, tail_split=4,
)


def _build_program():
    nc = bass.Bass()
    t_text = nc.dram_tensor("text", [BLOC, M, JX, D], BF16, kind="ExternalInput")
    # packed per-batch params: cols [0:65]=wq3aug [128 rows], [65:194]=qnaug
    # [rows 0:64 = [qn | ones]]
    t_pk = nc.dram_tensor("packed", [BLOC, D, D + JQ + 3], BF16, kind="ExternalInput")
    t_q2 = nc.dram_tensor("q2aug", [BLOC, JQ + 1, 1], F32, kind="ExternalInput")
    t_out = nc.dram_tensor("out", [BLOC, M, JX, 3 * D], FP16, kind="ExternalOutput")

    with tile.TileContext(nc) as tc:
        import contextlib
        ctx = contextlib.ExitStack()
        with ctx:
            singles = ctx.enter_context(tc.tile_pool(name="singles", bufs=1))
            perb = ctx.enter_context(tc.tile_pool(name="perb", bufs=2))
            ptext = ctx.enter_context(tc.tile_pool(name="ptext", bufs=CFG["ptext"]))
            ptextd = ctx.enter_context(tc.tile_pool(name="ptextd", bufs=CFG["ptextd"]))
            pet = ctx.enter_context(tc.tile_pool(name="pet", bufs=CFG["pet"]))
            po123 = ctx.enter_context(tc.tile_pool(name="po123", bufs=CFG["po123"]))
            psmall = ctx.enter_context(tc.tile_pool(name="psmall", bufs=CFG["psmall"]))
            ptabc = ctx.enter_context(tc.tile_pool(name="ptabc", bufs=CFG["ptabc"]))
            ps_ttp = ctx.enter_context(tc.tile_pool(name="ps_ttp", bufs=CFG["ttp"], space="PSUM"))
            ps_cross = ctx.enter_context(tc.tile_pool(name="ps_cross", bufs=CFG["cross"], space="PSUM"))
            ps_etr = ctx.enter_context(tc.tile_pool(name="ps_etr", bufs=CFG["etr"], space="PSUM"))
            ps_tabc = ctx.enter_context(tc.tile_pool(name="ps_tabc", bufs=CFG["tabc"], space="PSUM"))
            ps_attnu = ctx.enter_context(tc.tile_pool(name="ps_attnu", bufs=CFG["attnu"], space="PSUM"))

            # issue the very first text load before any constant setup so the
            # DMA engines start immediately
            first_text = ptext.tile([128, NT * D], BF16, tag="text")
            _fsrc = t_text[0, 0].rearrange("(t p k) d -> p t k d", p=128, k=2)
            getattr(nc, CFG["q_tin"]).dma_start(
                out=first_text.rearrange("p (t k d) -> p t k d", t=NTH, k=2),
                in_=_fsrc)

            identb = singles.tile([128, 128], BF16)
            make_identity(nc, identb)
            identb65 = singles.tile([JQ + 1, JQ + 1], BF16)
            make_identity(nc, identb65)
            ones128 = singles.tile([128, 128], BF16)
            nc.vector.memset(ones128, 1.0)

            def prep_text(gb, m):
                """Load + transpose + PSUM->SBUF copy of one text unit.
                Issued one unit AHEAD of the consuming compute so the ACT
                textd copy does not sit between exp(n) and cross(n+1) on the
                in-order ACT queue (that cycle gates the whole pipeline)."""
                # partition p, block u=2t+k holds DRAM row i=256t+2p+k
                if gb == 0 and m == 0:
                    text_il = first_text
                else:
                    text_il = ptext.tile([128, NT * D], BF16, tag="text")
                    src = t_text[gb, m].rearrange(
                        "(t p k) d -> p t k d", p=128, k=2)
                    dst = text_il.rearrange(
                        "p (t k d) -> p t k d", t=NTH, k=2)
                    nsi = CFG["split_in"]
                    for h in range(nsi):
                        hh = NTH // nsi
                        getattr(nc, CFG["q_tin"]).dma_start(
                            out=dst[:, h * hh:(h + 1) * hh],
                            in_=src[:, h * hh:(h + 1) * hh])
                ttp = ps_ttp.tile([128, JX], BF16, tag="ttp")
                for u in range(NT):
                    nc.tensor.transpose(
                        ttp[:, u * 128:(u + 1) * 128],
                        text_il[:, u * D:(u + 1) * D], identb)
                textd = ptextd.tile([128, JX], BF16, tag="textd")
                if CFG["eng_textd"] == "act":
                    nc.scalar.copy(out=textd, in_=ttp)
                else:
                    nc.vector.tensor_copy(textd, ttp)
                return text_il, textd

            pending = prep_text(0, 0)
            for gb in range(BLOC):
                pk_sb = perb.tile([D, D + JQ + 3], BF16, tag="pk")
                q2_sb = perb.tile([JQ + 1, 1], F32, tag="q2")
                qd = getattr(nc, CFG["q_small"])
                qd.dma_start(out=pk_sb, in_=t_pk[gb])
                qd.dma_start(out=q2_sb, in_=t_q2[gb])
                wq3_sb = pk_sb[:, 0:JQ + 1]
                qn_sb = pk_sb[0:JQ, JQ + 1: JQ + 1 + D + 1]

                for m in range(M):
                    unit = gb * M + m
                    text_il, textd = pending
                    text3 = text_il.rearrange("p (u d) -> p u d", d=D)

                    # ---- crossT_aug = [w3q|w1].T @ text_d  [65, 512] ----
                    cross = ps_cross.tile([JQ + 1, JX], F32, tag="cross")
                    nc.tensor.matmul(cross, wq3_sb, textd, start=True, stop=True)

                    # ---- eT = exp(cross + q2) (row 64 = exp(t1)) ----
                    eT = pet.tile([JQ + 1, JX], BF16, tag="eT")
                    nc.scalar.activation(
                        out=eT, in_=cross,
                        func=mybir.ActivationFunctionType.Exp,
                        bias=q2_sb[:, 0:1], scale=1.0)

                    # prep the NEXT unit now: its ACT copy lands after exp(n)
                    # and its PE transposes after cross(n), so neither blocks
                    # the cross->exp critical cycle
                    if unit + 1 < BLOC * M:
                        pending = prep_text((unit + 1) // M, (unit + 1) % M)

                    # ---- transpose eT slices -> etr [128, 4*65] ----
                    etr = ps_etr.tile([128, NT * (JQ + 1)], BF16, tag="etr")
                    for u in range(NT):
                        nc.tensor.transpose(
                            etr[:, u * (JQ + 1):(u + 1) * (JQ + 1)],
                            eT[:, u * 128:(u + 1) * 128], identb65)
                    etr_blk = etr[:, :].rearrange("p (u j) -> p u j", j=JQ + 1)

                    # ---- qlmax path: etq = exp(qlmax) = G * exp(t1) ----
                    gq = psmall.tile([128, NT], BF16, tag="gq")
                    nc.vector.tensor_reduce(
                        out=gq, in_=etr_blk[:, :, 0:JQ],
                        axis=mybir.AxisListType.X, op=mybir.AluOpType.max)
                    etq = psmall.tile([128, NT], BF16, tag="etq")
                    nc.vector.tensor_mul(etq, gq, etr_blk[:, :, JQ])

                    # ---- attnu = eT[0:64].T @ qn; qa = attnu*rq ----
                    # tabu regions: [0:D] text_attn bcast, [D] Zt bcast,
                    # [D+1:D+1+NT] attnu softmax denominators Z(i)
                    tabu = ps_tabc.tile([128, D + 1 + NT], F32, tag="tabu")
                    o123 = po123.tile([128, NT, 3 * D], FP16, tag="o123")
                    if CFG["qa_merge"]:
                        onesq = pk_sb[0:JQ, JQ + 1 + D:JQ + 2 + D]
                        attnu = ps_attnu.tile([128, NT * D], F32, tag="attnu")
                        for u in range(NT):
                            nc.tensor.matmul(
                                attnu[:, u * D:(u + 1) * D],
                                eT[0:JQ, u * 128:(u + 1) * 128],
                                qn_sb[:, 0:D], start=True, stop=True)
                        for u in range(NT):
                            nc.tensor.matmul(
                                tabu[:, D + 1 + u:D + 2 + u],
                                eT[0:JQ, u * 128:(u + 1) * 128],
                                onesq, start=True, stop=True)
                        # Zt on every partition: sum_u sum_p etq[p,u], via
                        # accumulating 1-column matmuls (etq column bcast
                        # stationary, ones column moving) — issued here so
                        # ONE reciprocal covers [Zt | Z(i) x4]
                        for u in range(NT):
                            nc.tensor.matmul(
                                tabu[:, D:D + 1],
                                _col_bcast(etq[:, u:u + 1], 128),
                                ones128[:, 0:1],
                                start=(u == 0), stop=(u == NT - 1))
                        rqz = psmall.tile([128, NT + 1], F32, tag="rqz")
                        nc.vector.reciprocal(
                            out=rqz, in_=tabu[:, D:D + 1 + NT])
                        rq = rqz[:, 1:NT + 1]
                        rzt = rqz[:, 0:1]
                        nc.vector.tensor_tensor(
                            out=o123[:, :, 0:D],
                            in0=attnu.rearrange("p (u d) -> p u d", d=D),
                            in1=_bcast(rq, D, 2),
                            op=mybir.AluOpType.mult)
                    else:
                        raise NotImplementedError("qa_merge=False removed")

                    # ---- text_attn broadcast: every PE column = etq ----
                    for u in range(NT):
                        nc.tensor.matmul(
                            tabu[:, 0:D],
                            _col_bcast(etq[:, u:u + 1], 128),
                            text_il[:, u * D:(u + 1) * D],
                            start=(u == 0), stop=(u == NT - 1))
                    tabc = ptabc.tile([128, D], BF16, tag="tabc")
                    if CFG["eng_tabc"] == "act":
                        nc.scalar.mul(out=tabc, in_=tabu[:, 0:D], mul=rzt)
                    else:
                        nc.vector.tensor_scalar_mul(
                            out=tabc, in0=tabu[:, 0:D], scalar1=rzt)

                    # ---- col2 = text*qa, col3 = text*text_attn; store ----
                    for (cl, cu), dve_u in (
                            ((D, 2 * D), NT - CFG["col2_pool_u"]),
                            ((2 * D, 3 * D), NT - CFG["col3_pool_u"])):
                        for eng, u0, u1 in ((nc.vector, 0, dve_u),
                                            (nc.gpsimd, dve_u, NT)):
                            if u1 <= u0:
                                continue
                            in1 = (o123[:, u0:u1, 0:D] if cl == D else
                                   _bcast(tabc[:, :], u1 - u0, 1))
                            eng.tensor_mul(
                                o123[:, u0:u1, cl:cu],
                                text3[:, u0:u1, :], in1)
                    nsp = CFG["split_out"]
                    if BLOC * M - unit <= CFG["tail_split"]:
                        nsp = max(nsp, 2)
                    ht = NT // nsp
                    dst4 = t_out[gb, m].rearrange(
                        "(t p k) c -> p t k c", p=128, k=2)
                    o1234 = o123[:, :, :].rearrange(
                        "p (t k) c -> p t k c", k=2)
                    for h in range(nsp):
                        ts0, ts1 = h * ht, (h + 1) * ht
                        getattr(nc, CFG["q_out"]).dma_start(
                            out=dst4[:, ts0 // 2:ts1 // 2],
                            in_=o1234[:, ts0 // 2:ts1 // 2])

    _split_multi_waits(nc)
    return nc


_NC_CACHE = {}


def _get_nc():
    if "nc" not in _NC_CACHE:
        _NC_CACHE["nc"] = _build_program()
    return _NC_CACHE["nc"]


def _make_in_maps(text, query, w):
    w1, w2, w3 = w[:D], w[D:2 * D], w[2 * D:]
    in_maps = []
    for c in range(NCORES):
        sl = slice(c * BLOC, (c + 1) * BLOC)
        q = query[sl]                                    # [BLOC, 64, 128]
        q2 = np.concatenate(
            [np.einsum("bjd,d->bj", q, w2),
             np.zeros((BLOC, 1), np.float32)], axis=1)[:, :, None]
        # packed [D, 65 + 129 + 1]: [0:65]=wq3aug; rows 0:64 of [65:194] =
        # [qn | ones]; col 194 pad (keeps row length odd->even alignment)
        pk = np.zeros((BLOC, D, D + JQ + 3), np.float32)
        pk[:, :, 0:JQ] = np.einsum("bjd->bdj", q * w3[None, None, :])
        pk[:, :, JQ] = w1[None, :]
        pk[:, 0:JQ, JQ + 1:JQ + 1 + D] = q
        pk[:, 0:JQ, JQ + 1 + D] = 1.0
        m = {
            "text": np.ascontiguousarray(text[sl]).astype(ml_dtypes.bfloat16),
            "packed": np.ascontiguousarray(pk).astype(ml_dtypes.bfloat16),
            "q2aug": np.ascontiguousarray(q2, dtype=np.float32),
        }
        in_maps.append(m)
    return in_maps


def kernel(text, query, text_mask, query_mask, w, b, _want_results=False):
    text = np.asarray(text, dtype=np.float32)
    query = np.asarray(query, dtype=np.float32)
    w = np.asarray(w, dtype=np.float32)
    nc = _get_nc()
    in_maps = _make_in_maps(text, query, w)
    res = run_bass_kernel_spmd(nc, in_maps, core_ids=list(range(NCORES)))
    out = np.empty((B, M, JX, 4 * D), dtype=np.float32)
    out[..., 0:D] = text
    for c in range(NCORES):
        out[c * BLOC:(c + 1) * BLOC, ..., D:] = res.results[c]["out"]
    if _want_results:
        return out, res
    return out


# revision 39
# speedup vs baseline: 1.1979x; 1.0879x over previous
"""BiAttention (BiDAF-style) Trainium2 kernel — 8-core SPMD, memory-bound.

Contract: kernel(**inputs) takes the FULL tensors
  text [32,8,512,128] f32, query [32,64,128] f32, text_mask [32,8,512],
  query_mask [32,64], w [384], b [1]
and returns attn [32,8,512,512] f32, matching the reference

  w1,w2,w3 = w[:128], w[128:256], w[256:]
  logits[b,m,i,j] = text[b,m,i]·(w3*query[b,j]) + t1[b,m,i] + q2[b,j] + b
  p_q   = softmax_j logits      -> query_attn = p_q @ query
  qlmax = max_j logits          -> p_text = softmax_i qlmax
  text_attn = sum_i p_text*text
  out = concat([text, query_attn, text*query_attn, text*text_attn], -1)

The masks are all ones per the problem spec, so the (1-mask)*VERY_NEG term is
identically zero; bias b and the per-row t1 shift cancel inside softmax_j
(t1 is carried exactly via an extra lhsT column for the qlmax path).

v3 — HBM-traffic-minimized (16.9 MB/core vs 41.9 baseline):
  * output block 0 is a verbatim copy of the input `text`; the host fills it
    during unshard; the device stores only [query_attn, text*query_attn,
    text*text_attn] in fp16.
  * text is loaded bf16 with a paired-row interleave (partition p holds rows
    {256t+2p, 256t+2p+1}) keeping every DMA descriptor >= 512 B contiguous.
  * all matmuls bf16 (1 PE col/cycle).
  * qn carries an appended ones column so the attnu matmul also emits the
    softmax_j denominators Z(i) per-partition.
  * text_attn: stride-0-broadcast stationary (every PE column = etq) makes
    the weighted-sum matmul emit its result broadcast across all 128
    partitions; one fused DVE tensor_scalar normalizes+casts it to bf16.
  * elementwise work is split across DVE / ACT / Pool per CFG.

Sharding: batch B=32 data-parallel across 8 NeuronCores (BLOC=4 per core),
32 (b,m) units per core, no collectives.  Host precomputes O(query)-sized
helpers only (packed into one bf16 tile per batch + tiny f32 q2).

Toolchain note: walrus in this container encodes ONE sync-wait per
instruction; _split_multi_waits() legalizes the Tile-emitted program.
"""

import os
import sys

for _p in ("/opt/trn_rl_repo", "/root/.axon_site/_ro/trn_rl_repo"):
    if os.path.isdir(_p) and _p not in sys.path:
        sys.path.insert(0, _p)

import numpy as np
import ml_dtypes

import concourse.bass as bass
import concourse.tile as tile
from concourse import mybir
from concourse.bass_utils import run_bass_kernel_spmd
from concourse.masks import make_identity

NCORES = 8
B, M, JX, JQ, D = 32, 8, 512, 64, 128
BLOC = B // NCORES          # batches per core
NT = JX // 128              # 128-col i-blocks per (b,m)
NTH = NT // 2               # paired-row DMA t-blocks
F32 = mybir.dt.float32
BF16 = mybir.dt.bfloat16
FP16 = mybir.dt.float16


def _split_multi_waits(nc):
    """walrus encodes one sync-wait per instruction; Tile may attach several.
    Split the extras into standalone EventSemaphore (sequencer wait)
    instructions placed directly before the instruction on the same engine."""
    n = 0
    for fn in nc.m.functions:
        for bb in fn.blocks:
            out = []
            for inst in bb.instructions:
                si = inst.sync_info
                if si is not None and si.on_wait and len(si.on_wait) > 1:
                    waits = list(si.on_wait)
                    for k, w in enumerate(waits[:-1]):
                        out.append(mybir.InstEventSemaphore(
                            name=f"{inst.name}-sw{k}",
                            engine=inst.engine,
                            ins=[], outs=[],
                            sync_info=mybir.SyncInfo(on_wait=[w], on_update=[]),
                        ))
                        n += 1
                    inst.sync_info = mybir.SyncInfo(
                        on_wait=[waits[-1]], on_update=list(si.on_update))
                out.append(inst)
            bb.instructions = out
    return n


def _bcast(ap, reps, axis):
    """Stride-0 broadcast AP: insert [0, reps] at `axis` of ap's dims."""
    a = [list(d) for d in ap.ap]
    a.insert(axis, [0, reps])
    return bass.AP(tensor=ap.tensor, offset=ap.offset, ap=a)


def _col_bcast(ap_col, reps):
    """[128,1] column AP -> [128, reps] stride-0 stationary broadcast."""
    return bass.AP(tensor=ap_col.tensor, offset=ap_col.offset,
                   ap=[list(ap_col.ap[0]), [0, reps]])


CFG = dict(
    eng_textd="act",    # transposed-text PSUM->SBUF copy: act|dve
    eng_tabc="act",     # tabc normalize+cast: act|dve
    qa_merge=True,      # Z cols via separate tiny matmuls; single qa op
    merge_recip=False,  # one reciprocal for [Zt | Z(i)] vs two
    col2_pool_u=2,      # how many of the 4 col2 u-blocks run on Pool
    col3_pool_u=4,      # how many of the 4 col3 u-blocks run on Pool
    q_tin="sync", q_out="sync", q_small="scalar",
    ptext=16, ptextd=3, pet=4, po123=16, psmall=12, ptabc=4,
    ttp=2, cross=1, etr=1, attnu=2, tabc=2,
    split_in=1, split_out=1, tail_split=4, tail_cols_dve=2,
)


def _build_program():
    nc = bass.Bass()
    t_text = nc.dram_tensor("text", [BLOC, M, JX, D], BF16, kind="ExternalInput")
    # packed per-batch params: cols [0:65]=wq3aug [128 rows], [65:194]=qnaug
    # [rows 0:64 = [qn | ones]]
    t_pk = nc.dram_tensor("packed", [BLOC, D, D + JQ + 3], BF16, kind="ExternalInput")
    t_q2 = nc.dram_tensor("q2aug", [BLOC, JQ + 1, 1], F32, kind="ExternalInput")
    t_out = nc.dram_tensor("out", [BLOC, M, JX, 3 * D], FP16, kind="ExternalOutput")

    with tile.TileContext(nc) as tc:
        import contextlib
        ctx = contextlib.ExitStack()
        with ctx:
            singles = ctx.enter_context(tc.tile_pool(name="singles", bufs=1))
            perb = ctx.enter_context(tc.tile_pool(name="perb", bufs=2))
            ptext = ctx.enter_context(tc.tile_pool(name="ptext", bufs=CFG["ptext"]))
            ptextd = ctx.enter_context(tc.tile_pool(name="ptextd", bufs=CFG["ptextd"]))
            pet = ctx.enter_context(tc.tile_pool(name="pet", bufs=CFG["pet"]))
            po123 = ctx.enter_context(tc.tile_pool(name="po123", bufs=CFG["po123"]))
            psmall = ctx.enter_context(tc.tile_pool(name="psmall", bufs=CFG["psmall"]))
            ptabc = ctx.enter_context(tc.tile_pool(name="ptabc", bufs=CFG["ptabc"]))
            ps_ttp = ctx.enter_context(tc.tile_pool(name="ps_ttp", bufs=CFG["ttp"], space="PSUM"))
            ps_cross = ctx.enter_context(tc.tile_pool(name="ps_cross", bufs=CFG["cross"], space="PSUM"))
            ps_etr = ctx.enter_context(tc.tile_pool(name="ps_etr", bufs=CFG["etr"], space="PSUM"))
            ps_tabc = ctx.enter_context(tc.tile_pool(name="ps_tabc", bufs=CFG["tabc"], space="PSUM"))
            ps_attnu = ctx.enter_context(tc.tile_pool(name="ps_attnu", bufs=CFG["attnu"], space="PSUM"))

            # issue the very first text load before any constant setup so the
            # DMA engines start immediately
            first_text = ptext.tile([128, NT * D], BF16, tag="text")
            _fsrc = t_text[0, 0].rearrange("(t p k) d -> p t k d", p=128, k=2)
            getattr(nc, CFG["q_tin"]).dma_start(
                out=first_text.rearrange("p (t k d) -> p t k d", t=NTH, k=2),
                in_=_fsrc)

            identb = singles.tile([128, 128], BF16)
            make_identity(nc, identb)
            identb65 = singles.tile([JQ + 1, JQ + 1], BF16)
            make_identity(nc, identb65)
            ones128 = singles.tile([128, 128], BF16)
            nc.vector.memset(ones128, 1.0)

            def prep_text(gb, m):
                """Load + transpose + PSUM->SBUF copy of one text unit.
                Issued one unit AHEAD of the consuming compute so the ACT
                textd copy does not sit between exp(n) and cross(n+1) on the
                in-order ACT queue (that cycle gates the whole pipeline)."""
                # partition p, block u=2t+k holds DRAM row i=256t+2p+k
                if gb == 0 and m == 0:
                    text_il = first_text
                else:
                    text_il = ptext.tile([128, NT * D], BF16, tag="text")
                    src = t_text[gb, m].rearrange(
                        "(t p k) d -> p t k d", p=128, k=2)
                    dst = text_il.rearrange(
                        "p (t k d) -> p t k d", t=NTH, k=2)
                    nsi = CFG["split_in"]
                    for h in range(nsi):
                        hh = NTH // nsi
                        getattr(nc, CFG["q_tin"]).dma_start(
                            out=dst[:, h * hh:(h + 1) * hh],
                            in_=src[:, h * hh:(h + 1) * hh])
                ttp = ps_ttp.tile([128, JX], BF16, tag="ttp")
                for u in range(NT):
                    nc.tensor.transpose(
                        ttp[:, u * 128:(u + 1) * 128],
                        text_il[:, u * D:(u + 1) * D], identb)
                textd = ptextd.tile([128, JX], BF16, tag="textd")
                if CFG["eng_textd"] == "act":
                    nc.scalar.copy(out=textd, in_=ttp)
                else:
                    nc.vector.tensor_copy(textd, ttp)
                return text_il, textd

            pending = prep_text(0, 0)
            for gb in range(BLOC):
                pk_sb = perb.tile([D, D + JQ + 3], BF16, tag="pk")
                q2_sb = perb.tile([JQ + 1, 1], F32, tag="q2")
                qd = getattr(nc, CFG["q_small"])
                qd.dma_start(out=pk_sb, in_=t_pk[gb])
                qd.dma_start(out=q2_sb, in_=t_q2[gb])
                wq3_sb = pk_sb[:, 0:JQ + 1]
                qn_sb = pk_sb[0:JQ, JQ + 1: JQ + 1 + D + 1]

                for m in range(M):
                    unit = gb * M + m
                    text_il, textd = pending
                    text3 = text_il.rearrange("p (u d) -> p u d", d=D)

                    # ---- crossT_aug = [w3q|w1].T @ text_d  [65, 512] ----
                    cross = ps_cross.tile([JQ + 1, JX], F32, tag="cross")
                    nc.tensor.matmul(cross, wq3_sb, textd, start=True, stop=True)

                    # ---- eT = exp(cross + q2) (row 64 = exp(t1)) ----
                    eT = pet.tile([JQ + 1, JX], BF16, tag="eT")
                    nc.scalar.activation(
                        out=eT, in_=cross,
                        func=mybir.ActivationFunctionType.Exp,
                        bias=q2_sb[:, 0:1], scale=1.0)

                    # prep the NEXT unit now: its ACT copy lands after exp(n)
                    # and its PE transposes after cross(n), so neither blocks
                    # the cross->exp critical cycle
                    if unit + 1 < BLOC * M:
                        pending = prep_text((unit + 1) // M, (unit + 1) % M)

                    # ---- transpose eT slices -> etr [128, 4*66] ----
                    # (66-col stride keeps each bf16 block 4-byte aligned
                    # in PSUM; col 65 of each block is pad)
                    JB = JQ + 2
                    etr = ps_etr.tile([128, NT * JB], BF16, tag="etr")
                    for u in range(NT):
                        nc.tensor.transpose(
                            etr[:, u * JB:u * JB + JQ + 1],
                            eT[:, u * 128:(u + 1) * 128], identb65)
                    etr_blk = etr[:, :].rearrange("p (u j) -> p u j", j=JB)

                    # ---- qlmax path: etq = exp(qlmax) = G * exp(t1) ----
                    gq = psmall.tile([128, NT], BF16, tag="gq")
                    nc.vector.tensor_reduce(
                        out=gq, in_=etr_blk[:, :, 0:JQ],
                        axis=mybir.AxisListType.X, op=mybir.AluOpType.max)
                    etq = psmall.tile([128, NT], BF16, tag="etq")
                    nc.vector.tensor_mul(etq, gq, etr_blk[:, :, JQ])

                    # ---- attnu = eT[0:64].T @ qn; qa = attnu*rq ----
                    # tabu regions: [0:D] text_attn bcast, [D] Zt bcast,
                    # [D+1:D+1+NT] attnu softmax denominators Z(i)
                    tabu = ps_tabc.tile([128, D + 1 + NT], F32, tag="tabu")
                    o123 = po123.tile([128, NT, 3 * D], FP16, tag="o123")
                    if CFG["qa_merge"]:
                        onesq = pk_sb[0:JQ, JQ + 1 + D:JQ + 2 + D]
                        attnu = ps_attnu.tile([128, NT * D], F32, tag="attnu")
                        for u in range(NT):
                            nc.tensor.matmul(
                                attnu[:, u * D:(u + 1) * D],
                                eT[0:JQ, u * 128:(u + 1) * 128],
                                qn_sb[:, 0:D], start=True, stop=True)
                        for u in range(NT):
                            nc.tensor.matmul(
                                tabu[:, D + 1 + u:D + 2 + u],
                                eT[0:JQ, u * 128:(u + 1) * 128],
                                onesq, start=True, stop=True)
                        if CFG["merge_recip"]:
                            # Zt via accumulating 1-column matmuls (etq col
                            # bcast stationary, ones col moving) issued here
                            # so ONE reciprocal covers [Zt | Z(i) x4]
                            for u in range(NT):
                                nc.tensor.matmul(
                                    tabu[:, D:D + 1],
                                    _col_bcast(etq[:, u:u + 1], 128),
                                    ones128[:, 0:1],
                                    start=(u == 0), stop=(u == NT - 1))
                            rqz = psmall.tile([128, NT + 1], F32, tag="rqz")
                            nc.vector.reciprocal(
                                out=rqz, in_=tabu[:, D:D + 1 + NT])
                            rq = rqz[:, 1:NT + 1]
                            rzt = rqz[:, 0:1]
                        else:
                            rq = psmall.tile([128, NT], F32, tag="rq")
                            nc.vector.reciprocal(
                                out=rq, in_=tabu[:, D + 1:D + 1 + NT])
                            rzt = None
                        nc.vector.tensor_tensor(
                            out=o123[:, :, 0:D],
                            in0=attnu.rearrange("p (u d) -> p u d", d=D),
                            in1=_bcast(rq, D, 2),
                            op=mybir.AluOpType.mult)
                    else:
                        raise NotImplementedError("qa_merge=False removed")

                    # ---- text_attn broadcast: every PE column = etq ----
                    for u in range(NT):
                        nc.tensor.matmul(
                            tabu[:, 0:D],
                            _col_bcast(etq[:, u:u + 1], 128),
                            text_il[:, u * D:(u + 1) * D],
                            start=(u == 0), stop=(u == NT - 1))
                    if rzt is None:
                        for u in range(NT):
                            nc.tensor.matmul(
                                tabu[:, D:D + 1],
                                _col_bcast(etq[:, u:u + 1], 128),
                                ones128[:, 0:1],
                                start=(u == 0), stop=(u == NT - 1))
                        rzt = psmall.tile([128, 1], F32, tag="rzt")
                        nc.vector.reciprocal(out=rzt, in_=tabu[:, D:D + 1])
                    tabc = ptabc.tile([128, D], BF16, tag="tabc")
                    if CFG["eng_tabc"] == "act":
                        nc.scalar.mul(out=tabc, in_=tabu[:, 0:D], mul=rzt)
                    else:
                        nc.vector.tensor_scalar_mul(
                            out=tabc, in0=tabu[:, 0:D], scalar1=rzt)

                    # ---- col2 = text*qa, col3 = text*text_attn; store ----
                    tail = BLOC * M - unit <= CFG["tail_cols_dve"]
                    for (cl, cu), dve_u in (
                            ((D, 2 * D),
                             NT if tail else NT - CFG["col2_pool_u"]),
                            ((2 * D, 3 * D),
                             NT if tail else NT - CFG["col3_pool_u"])):
                        for eng, u0, u1 in ((nc.vector, 0, dve_u),
                                            (nc.gpsimd, dve_u, NT)):
                            if u1 <= u0:
                                continue
                            in1 = (o123[:, u0:u1, 0:D] if cl == D else
                                   _bcast(tabc[:, :], u1 - u0, 1))
                            eng.tensor_mul(
                                o123[:, u0:u1, cl:cu],
                                text3[:, u0:u1, :], in1)
                    nsp = CFG["split_out"]
                    if BLOC * M - unit <= CFG["tail_split"]:
                        nsp = max(nsp, 2)
                    ht = NT // nsp
                    dst4 = t_out[gb, m].rearrange(
                        "(t p k) c -> p t k c", p=128, k=2)
                    o1234 = o123[:, :, :].rearrange(
                        "p (t k) c -> p t k c", k=2)
                    for h in range(nsp):
                        ts0, ts1 = h * ht, (h + 1) * ht
                        getattr(nc, CFG["q_out"]).dma_start(
                            out=dst4[:, ts0 // 2:ts1 // 2],
                            in_=o1234[:, ts0 // 2:ts1 // 2])

    _split_multi_waits(nc)
    return nc


_NC_CACHE = {}


def _get_nc():
    if "nc" not in _NC_CACHE:
        _NC_CACHE["nc"] = _build_program()
    return _NC_CACHE["nc"]


def _make_in_maps(text, query, w):
    w1, w2, w3 = w[:D], w[D:2 * D], w[2 * D:]
    in_maps = []
    for c in range(NCORES):
        sl = slice(c * BLOC, (c + 1) * BLOC)
        q = query[sl]                                    # [BLOC, 64, 128]
        q2 = np.concatenate(
            [np.einsum("bjd,d->bj", q, w2),
             np.zeros((BLOC, 1), np.float32)], axis=1)[:, :, None]
        # packed [D, 65 + 129 + 1]: [0:65]=wq3aug; rows 0:64 of [65:194] =
        # [qn | ones]; col 194 pad (keeps row length odd->even alignment)
        pk = np.zeros((BLOC, D, D + JQ + 3), np.float32)
        pk[:, :, 0:JQ] = np.einsum("bjd->bdj", q * w3[None, None, :])
        pk[:, :, JQ] = w1[None, :]
        pk[:, 0:JQ, JQ + 1:JQ + 1 + D] = q
        pk[:, 0:JQ, JQ + 1 + D] = 1.0
        m = {
            "text": np.ascontiguousarray(text[sl]).astype(ml_dtypes.bfloat16),
            "packed": np.ascontiguousarray(pk).astype(ml_dtypes.bfloat16),
            "q2aug": np.ascontiguousarray(q2, dtype=np.float32),
        }
        in_maps.append(m)
    return in_maps


def kernel(text, query, text_mask, query_mask, w, b, _want_results=False):
    text = np.asarray(text, dtype=np.float32)
    query = np.asarray(query, dtype=np.float32)
    w = np.asarray(w, dtype=np.float32)
    nc = _get_nc()
    in_maps = _make_in_maps(text, query, w)
    res = run_bass_kernel_spmd(nc, in_maps, core_ids=list(range(NCORES)))
    out = np.empty((B, M, JX, 4 * D), dtype=np.float32)
    out[..., 0:D] = text
    for c in range(NCORES):
        out[c * BLOC:(c + 1) * BLOC, ..., D:] = res.results[c]["out"]
    if _want_results:
        return out, res
    return out
